# revision 1
# baseline (speedup 1.0000x reference)
"""Trainium2 Bass kernel for nn_EnergyPredictor (segment-softmax attention pooling).

Math: for each of 3 row sets (rec nodes, lig nodes, cross edges) compute
per-row scores s = relu(x@W1+b1)@W2+b2, then attention-pool per graph
segment.  Scores here are tiny (|s| < ~1), so softmax needs no
max-subtraction and the pooled value is computed in ONE pass over x:

    pooled[g] = sum_{i in g} x_i * exp(s_i)  /  sum_{i in g} exp(s_i)

Per 128-row tile the segment reduction is a one-hot matmul:
    lhsT = onehot(idx)*exp(s)  [128 rows, 64 graphs]
    rhs  = [x | 1]             [128 rows, 257]
    acc[64, 257] += lhsT.T @ rhs   (PSUM accumulation across all tiles)
so column 256 of acc is the softmax denominator.

Sharding: rows of the 3 sets are split evenly over 8 NeuronCores; each
core computes partial [64, 3*257] sums, an on-device AllReduce combines
them, and every core runs the tiny 64-row energy MLP (replicated).
Core 0's output is returned.
"""

import math
import numpy as np

P = 128            # partitions
G = 64             # num graphs
FOLD = 256
HALF = 128
NPG = 4            # 128-row chunks per group
RG = P * NPG       # rows per group = 512
NCORES = 8

_POOLS = ("rec", "lig", "cross")

_cache = {}

# tunable buffer counts (model-sweep via tl_analyze)
BUFS = dict(xs=8, xt=8, rh=6, small=6, pt=3, ph=2, ps=2)
RELU_ENGINE = "dve"  # "act" | "dve" | "both" (half each)
COPIES = "split"     # "split": t0 ACT/t1 DVE; "act": both ACT
SPLIT_AR = True      # all-reduce rec+lig early (hides under cross loop)
GPD = 1              # 512-row groups per x DMA (2 -> 1MB transfers)
SCORE_MODE = "v1"  # "v1": per-chunk s-matmul (relu_hT stationary); "v3": W2 stationary


def _pad_rows(n_total):
    """rows per core, padded up to a multiple of GPD*RG"""
    per = math.ceil(n_total / NCORES)
    return math.ceil(per / (GPD * RG)) * (GPD * RG)


def _build(cfg, repeat=1, collective=True, ncores=NCORES):
    """cfg: dict pool -> padded rows per core. Returns compiled nc (hw module)."""
    import concourse.mybir as mybir
    import concourse.tile as tile
    from concourse import bacc
    from concourse.bass_interp import get_hw_module

    f32 = mybir.dt.float32
    f32r = mybir.dt.float32r
    AF = mybir.ActivationFunctionType
    OP = mybir.AluOpType

    nc = bacc.Bacc("TRN2", target_bir_lowering=False, debug=False,
                   num_devices=ncores)

    # ---------------- DRAM I/O ----------------
    xd, idxd, w1d, w2d, b1d, b2d = {}, {}, {}, {}, {}, {}
    for pl in _POOLS:
        R = cfg[pl]
        xd[pl] = nc.dram_tensor(f"x_{pl}", [R, FOLD + 2], f32r, kind="ExternalInput")
        idxd[pl] = nc.dram_tensor(f"idx_{pl}", [P, (R // RG) * NPG], f32,
                                  kind="ExternalInput")
        w1d[pl] = nc.dram_tensor(f"w1_{pl}", [FOLD, HALF], f32r, kind="ExternalInput")
        w2d[pl] = nc.dram_tensor(f"w2_{pl}", [HALF, 2], f32r, kind="ExternalInput")
        b1d[pl] = nc.dram_tensor(f"b1_{pl}", [HALF, 1], f32, kind="ExternalInput")
        b2d[pl] = nc.dram_tensor(f"b2_{pl}", [P, 1], f32, kind="ExternalInput")
    iotad = nc.dram_tensor("iota", [P, G], f32, kind="ExternalInput")
    identd = nc.dram_tensor("ident", [P, P], f32r, kind="ExternalInput")
    mw1d = nc.dram_tensor("mlp_w1", [3 * FOLD, FOLD], f32, kind="ExternalInput")
    mb1d = nc.dram_tensor("mlp_b1", [P, 2], f32, kind="ExternalInput")
    mw2d = nc.dram_tensor("mlp_w2", [FOLD, HALF], f32, kind="ExternalInput")
    mb2d = nc.dram_tensor("mlp_b2", [P, 1], f32, kind="ExternalInput")
    owd = nc.dram_tensor("out_w", [HALF, 1], f32, kind="ExternalInput")
    obd = nc.dram_tensor("out_b", [1, 1], f32, kind="ExternalInput")
    energyd = nc.dram_tensor("energy", [G, 1], f32, kind="ExternalOutput")

    with tile.TileContext(nc) as tc:
        with (
            tc.tile_pool(name="const", bufs=1) as const,
            tc.tile_pool(name="xs", bufs=BUFS["xs"]) as xs_pool,
            tc.tile_pool(name="xt", bufs=BUFS["xt"]) as xt_pool,
            tc.tile_pool(name="rh", bufs=BUFS["rh"]) as rh_pool,
            tc.tile_pool(name="small", bufs=BUFS["small"]) as small_pool,
            tc.tile_pool(name="fin", bufs=1) as fin_pool,
            tc.tile_pool(name="psum_acc", bufs=1, space="PSUM") as psum_acc,
            tc.tile_pool(name="psum_t", bufs=BUFS["pt"], space="PSUM") as psum_t,
            tc.tile_pool(name="psum_h", bufs=BUFS["ph"], space="PSUM") as psum_h,
            tc.tile_pool(name="psum_s", bufs=BUFS["ps"], space="PSUM") as psum_s,
            tc.tile_pool(name="dram", bufs=1, space="DRAM") as dram_pool,
        ):
            # ---------------- constants / weights ----------------
            iota_sb = const.tile([P, G], f32, tag="iota")
            nc.sync.dma_start(iota_sb[:], iotad.ap())
            ident_sb = const.tile([P, P], f32r, tag="ident")
            nc.sync.dma_start(ident_sb[:], identd.ap())

            w1_sb, w2_sb, b1_sb, b2_sb, idx_sb = {}, {}, {}, {}, {}
            for pl in _POOLS:
                w1_sb[pl] = const.tile([P, 2, HALF], f32r, tag=f"w1_{pl}", name=f"w1sb_{pl}")
                nc.sync.dma_start(
                    w1_sb[pl][:], w1d[pl].ap().rearrange("(c p) h -> p c h", p=P))
                w2_sb[pl] = const.tile([P, 2], f32r, tag=f"w2_{pl}", name=f"w2sb_{pl}")
                nc.sync.dma_start(w2_sb[pl][:], w2d[pl].ap())
                b1_sb[pl] = const.tile([P, 1], f32, tag=f"b1_{pl}", name=f"b1sb_{pl}")
                nc.sync.dma_start(b1_sb[pl][:], b1d[pl].ap())
                b2_sb[pl] = const.tile([P, 1], f32, tag=f"b2_{pl}", name=f"b2sb_{pl}")
                nc.sync.dma_start(b2_sb[pl][:], b2d[pl].ap())
                ncols = (cfg[pl] // RG) * NPG
                idx_sb[pl] = const.tile([P, ncols], f32, tag=f"idx_{pl}", name=f"idxsb_{pl}")
                nc.sync.dma_start(idx_sb[pl][:], idxd[pl].ap())

            mw1_sb = const.tile([P, 6, FOLD], f32, tag="mw1")
            nc.sync.dma_start(mw1_sb[:], mw1d.ap().rearrange("(k p) m -> p k m", p=P))
            mb1_sb = const.tile([P, 2], f32, tag="mb1")
            nc.sync.dma_start(mb1_sb[:], mb1d.ap())
            mw2_sb = const.tile([P, 2, HALF], f32, tag="mw2")
            nc.sync.dma_start(mw2_sb[:], mw2d.ap().rearrange("(m p) h -> p m h", p=P))
            mb2_sb = const.tile([P, 1], f32, tag="mb2")
            nc.sync.dma_start(mb2_sb[:], mb2d.ap())
            ow_sb = const.tile([P, 1], f32, tag="ow")
            nc.sync.dma_start(ow_sb[:], owd.ap())
            ob_sb = const.tile([1, 1], f32, tag="ob")
            nc.sync.dma_start(ob_sb[:], obd.ap())

                        # ---------------- main loops ----------------
            # (optionally repeated for K-diff timing; rep>0 output is identical)
            for _rep in range(repeat):
                accsb = fin_pool.tile([G, 3, FOLD + 1], f32, tag="accsb")

                for ipl, pl in enumerate(_POOLS):
                    acc = psum_acc.tile([G, FOLD + 2], f32, tag="acc", name=f"accps_{pl}")
                    ngroups = cfg[pl] // RG
                    assert ngroups % GPD == 0
                    x_ap = xd[pl].ap().rearrange("(g n p) d -> g p n d",
                                                 n=NPG * GPD, p=P)
                    n_acc = ngroups * NPG
                    xs_macro = None
                    for g in range(ngroups):
                        # load x group [128, NPG, 258] (ones col baked in);
                        # with GPD>1 one DMA covers GPD consecutive groups
                        if g % GPD == 0:
                            xs_macro = xs_pool.tile([P, NPG * GPD, FOLD + 2],
                                                    f32r, tag="xs", name="xs")
                            nc.sync.dma_start(xs_macro[:], x_ap[g // GPD])
                        xs = xs_macro[:, (g % GPD) * NPG:(g % GPD + 1) * NPG]

                        # transpose x -> xT chunks [128 feat, 512 rows] (per k half)
                        xt = []
                        for k in range(2):
                            tp = psum_t.tile([P, RG], f32r, tag="t")
                            for j in range(NPG):
                                nc.tensor.transpose(
                                    tp[:, j * P:(j + 1) * P],
                                    xs[:, j, k * P:(k + 1) * P],
                                    ident_sb[:])
                            xtk = xt_pool.tile([P, RG], f32r, tag=f"xt{k}")
                            if k == 0 or COPIES == "act":
                                nc.scalar.activation(xtk[:], tp[:], AF.Copy)
                            else:
                                nc.vector.tensor_copy(xtk[:], tp[:])
                            xt.append(xtk)

                        # hT [half=128, 512 rows] = W1.T @ xT  (accumulate 2 chunks)
                        hp = psum_h.tile([P, RG], f32, tag="h")
                        for k in range(2):
                            nc.tensor.matmul(hp[:], w1_sb[pl][:, k, :], xt[k][:],
                                             start=(k == 0), stop=(k == 1))
                        rh = rh_pool.tile([P, RG], f32r, tag="rh")
                        if RELU_ENGINE == "dve":
                            nc.vector.tensor_scalar(rh[:], hp[:], b1_sb[pl][:], 0.0,
                                                    OP.add, OP.max)
                        elif RELU_ENGINE == "both":
                            half = RG // 2
                            nc.vector.tensor_scalar(rh[:, :half], hp[:, :half],
                                                    b1_sb[pl][:], 0.0,
                                                    OP.add, OP.max)
                            nc.scalar.activation(rh[:, half:], hp[:, half:],
                                                 AF.Relu, bias=b1_sb[pl][:])
                        else:
                            nc.scalar.activation(rh[:], hp[:], AF.Relu,
                                                 bias=b1_sb[pl][:])

                        if SCORE_MODE == "v2":
                            # sT [2, 512] = W2.T @ relu_hT in one matmul with
                            # W2 stationary (prefetchable); exp on ACT; then
                            # tiny transposes put e back on the row axis.
                            stp = psum_s.tile([2, RG], f32, tag="st")
                            nc.tensor.matmul(stp[:], w2_sb[pl][:], rh[:],
                                             start=True, stop=True)
                            eT = small_pool.tile([2, RG], f32, tag="eT")
                            nc.scalar.activation(eT[:], stp[:], AF.Exp,
                                                 bias=b2_sb[pl][0:2, :])
                            etp = psum_s.tile([P, NPG * 2], f32, tag="s")
                            for j in range(NPG):
                                nc.tensor.transpose(
                                    etp[:, 2 * j:2 * j + 2],
                                    eT[:, j * P:(j + 1) * P],
                                    ident_sb[0:2, 0:2].bitcast(f32))
                            e = small_pool.tile([P, NPG * 2], f32, tag="e")
                            nc.vector.tensor_copy(e[:], etp[:])
                        elif SCORE_MODE == "v3":
                            # scores: sT chunks [2, 128] = W2.T @ relu_hT_j at
                            # partition offsets 0/32/64/96 (W2 stationary -> the
                            # weight load prefetches; no relu_hT weight reload)
                            stp = psum_s.tile([3 * 32 + 2, P], f32, tag="st")
                            for j in range(NPG):
                                nc.tensor.matmul(stp[32 * j:32 * j + 2, :],
                                                 w2_sb[pl][:],
                                                 rh[:, j * P:(j + 1) * P],
                                                 start=True, stop=True,
                                                 tile_position=(0, 32 * j))
                            ssb = small_pool.tile([3 * 32 + 2, P], f32, tag="ssb")
                            nc.scalar.activation(ssb[:], stp[:], AF.Copy)
                            # transpose scores back onto the row axis, then exp
                            etp = psum_s.tile([P, NPG * 2], f32, tag="s")
                            for j in range(NPG):
                                nc.tensor.transpose(
                                    etp[:, 2 * j:2 * j + 2],
                                    ssb[32 * j:32 * j + 2, :],
                                    ident_sb[32 * j:32 * j + 2,
                                             32 * j:32 * j + 2].bitcast(f32),
                                    tile_position=(32 * j, 0))
                            e = small_pool.tile([P, NPG * 2], f32, tag="e")
                            nc.scalar.activation(e[:], etp[:], AF.Exp,
                                                 bias=b2_sb[pl][:])
                        else:
                            sp = psum_s.tile([P, NPG * 2], f32, tag="s")
                            for j in range(NPG):
                                nc.tensor.matmul(sp[:, 2 * j:2 * j + 2],
                                                 rh[:, j * P:(j + 1) * P],
                                                 w2_sb[pl][:],
                                                 start=True, stop=True)
                            e = small_pool.tile([P, NPG * 2], f32, tag="e")
                            nc.scalar.activation(e[:], sp[:], AF.Exp,
                                                 bias=b2_sb[pl][:])

                        # one-hot * e  [128 rows, 64], then acc += onehot.T @ [x|1]
                        oh = small_pool.tile([P, NPG, G], f32r, tag="oh")
                        for j in range(NPG):
                            nc.vector.tensor_scalar(
                                oh[:, j], iota_sb[:],
                                idx_sb[pl][:, g * NPG + j:g * NPG + j + 1],
                                e[:, 2 * j:2 * j + 1], OP.is_equal, OP.mult)
                        for j in range(NPG):
                            it = g * NPG + j
                            nc.tensor.matmul(acc[:],
                                             oh[:, j], xs[:, j],
                                             start=(it == 0), stop=(it == n_acc - 1))

                    nc.vector.tensor_copy(accsb[:, ipl], acc[:, :FOLD + 1])

                    if collective and SPLIT_AR and ipl == 1:
                        # reduce rec+lig now; overlaps the cross loop
                        cc1_in = dram_pool.tile([G, 2 * (FOLD + 1)], f32,
                                                name="cc1_in")
                        cc1_out = dram_pool.tile([G, 2 * (FOLD + 1)], f32,
                                                 name="cc1_out")
                        nc.sync.dma_start(cc1_in[:], accsb[:, 0:2])
                        nc.gpsimd.collective_compute(
                            "AllReduce", mybir.AluOpType.add,
                            replica_groups=[list(range(ncores))],
                            ins=[cc1_in.opt()], outs=[cc1_out.opt()],
                        )

                # ---------------- all-reduce partials ----------------
                if collective and SPLIT_AR:
                    cc2_in = dram_pool.tile([G, FOLD + 1], f32, name="cc2_in")
                    cc2_out = dram_pool.tile([G, FOLD + 1], f32, name="cc2_out")
                    nc.sync.dma_start(cc2_in[:], accsb[:, 2])
                    nc.gpsimd.collective_compute(
                        "AllReduce", mybir.AluOpType.add,
                        replica_groups=[list(range(ncores))],
                        ins=[cc2_in.opt()], outs=[cc2_out.opt()],
                    )
                    red = fin_pool.tile([G, 3, FOLD + 1], f32, tag="red")
                    nc.sync.dma_start(red[:, 0:2], cc1_out[:])
                    nc.sync.dma_start(red[:, 2], cc2_out[:])
                elif collective:
                    cc_in = dram_pool.tile([G, 3 * (FOLD + 1)], f32)
                    cc_out = dram_pool.tile([G, 3 * (FOLD + 1)], f32)
                    nc.sync.dma_start(cc_in[:], accsb[:])
                    nc.gpsimd.collective_compute(
                        "AllReduce", mybir.AluOpType.add,
                        replica_groups=[list(range(ncores))],
                        ins=[cc_in.opt()], outs=[cc_out.opt()],
                    )
                    red = fin_pool.tile([G, 3, FOLD + 1], f32, tag="red")
                    nc.sync.dma_start(red[:], cc_out[:])
                else:
                    red = accsb

                # ---------------- pooled = N / D ; combined [64, 768] ----------------
                dsum = fin_pool.tile([G, 3], f32, tag="dsum")
                nc.vector.tensor_scalar(dsum[:], red[:, :, FOLD], 1e-30, None, OP.add)
                rcp = fin_pool.tile([G, 3], f32, tag="rcp")
                nc.vector.reciprocal(rcp[:], dsum[:])
                comb = fin_pool.tile([G, 3 * FOLD], f32, tag="comb")
                for i in range(3):
                    nc.vector.tensor_scalar(comb[:, i * FOLD:(i + 1) * FOLD],
                                            red[:, i, :FOLD], rcp[:, i:i + 1], None,
                                            OP.mult)

                # ---------------- energy MLP (fp32, replicated) ----------------
                combT = fin_pool.tile([P, 6, G], f32, tag="combT")
                for k in range(6):
                    tp = psum_t.tile([P, G], f32, tag="t")
                    nc.tensor.transpose(tp[:], comb[:, k * P:(k + 1) * P],
                                        ident_sb[:G, :G].bitcast(f32))
                    nc.scalar.activation(combT[:, k], tp[:], AF.Copy)

                r1 = fin_pool.tile([P, 2, G], f32, tag="r1")
                for m in range(2):
                    h1p = psum_h.tile([P, G], f32, tag="h")
                    for k in range(6):
                        nc.tensor.matmul(h1p[:], mw1_sb[:, k, m * P:(m + 1) * P],
                                         combT[:, k], start=(k == 0), stop=(k == 5))
                    nc.scalar.activation(r1[:, m], h1p[:], AF.Relu,
                                         bias=mb1_sb[:, m:m + 1])
                h2p = psum_h.tile([P, G], f32, tag="h")
                for m in range(2):
                    nc.tensor.matmul(h2p[:], mw2_sb[:, m], r1[:, m],
                                     start=(m == 0), stop=(m == 1))
                r2 = fin_pool.tile([P, G], f32, tag="r2")
                nc.scalar.activation(r2[:], h2p[:], AF.Relu, bias=mb2_sb[:])

                ep = psum_s.tile([1, G], f32, tag="s")
                nc.tensor.matmul(ep[:], ow_sb[:], r2[:], start=True, stop=True)
                en = fin_pool.tile([1, G], f32, tag="en")
                nc.vector.tensor_scalar(en[:], ep[:], ob_sb[:], None, OP.add)
                nc.sync.dma_start(energyd.ap(), en[:])

    nc.compile()
    nc.m = get_hw_module(nc.m)
    return nc


def _prep_pool(x, idx, n_pad):
    """shard rows of x/idx across cores, pad to n_pad rows per core.
    Returns per-core lists (x [n_pad,256] f32, idx [128, n_pad//128] f32)."""
    n = x.shape[0]
    per = math.ceil(n / NCORES)
    xs, idxs = [], []
    for c in range(NCORES):
        lo, hi = c * per, min((c + 1) * per, n)
        xp = np.zeros((n_pad, FOLD + 2), dtype=np.float32)
        xp[:, FOLD] = 1.0
        xp[:hi - lo, :FOLD] = x[lo:hi]
        ip = np.full((n_pad,), -1.0, dtype=np.float32)
        ip[:hi - lo] = idx[lo:hi]
        # rearrange (g n p) -> p (g n)
        ip = ip.reshape(n_pad // RG, NPG, P).transpose(2, 0, 1)
        xs.append(xp)
        idxs.append(np.ascontiguousarray(ip.reshape(P, -1)))
    return xs, idxs


def kernel(rec_na, lig_na, cross_ea, cross_idx, protein_batch, ligand_batch,
           num_graphs,
           pa_W1, pa_b1, pa_W2, pa_b2,
           la_W1, la_b1, la_W2, la_b2,
           ca_W1, ca_b1, ca_W2, ca_b2,
           mlp_W1, mlp_b1, mlp_W2, mlp_b2,
           out_W, out_b):
    from concourse import bass_utils

    assert int(num_graphs) == G
    rec_na = np.asarray(rec_na, dtype=np.float32)
    lig_na = np.asarray(lig_na, dtype=np.float32)
    cross_ea = np.asarray(cross_ea, dtype=np.float32)
    cross_idx = np.asarray(cross_idx)
    protein_batch = np.asarray(protein_batch)
    ligand_batch = np.asarray(ligand_batch)
    complex_ids = ligand_batch[cross_idx[0]]

    data = {
        "rec": (rec_na, protein_batch.astype(np.float32),
                pa_W1, pa_b1, pa_W2, pa_b2),
        "lig": (lig_na, ligand_batch.astype(np.float32),
                la_W1, la_b1, la_W2, la_b2),
        "cross": (cross_ea, complex_ids.astype(np.float32),
                  ca_W1, ca_b1, ca_W2, ca_b2),
    }
    cfg = {pl: _pad_rows(data[pl][0].shape[0]) for pl in _POOLS}

    key = tuple(sorted(cfg.items()))
    if key not in _cache:
        _cache[key] = _build(cfg)
    nc = _cache[key]

    shared = {
        "iota": np.broadcast_to(np.arange(G, dtype=np.float32), (P, G)).copy(),
        "ident": np.eye(P, dtype=np.float32),
        "mlp_w1": np.asarray(mlp_W1, dtype=np.float32),
        "mlp_b1": np.ascontiguousarray(
            np.asarray(mlp_b1, dtype=np.float32).reshape(2, P).T),
        "mlp_w2": np.asarray(mlp_W2, dtype=np.float32),
        "mlp_b2": np.asarray(mlp_b2, dtype=np.float32).reshape(P, 1),
        "out_w": np.asarray(out_W, dtype=np.float32),
        "out_b": np.asarray(out_b, dtype=np.float32).reshape(1, 1),
    }
    percore = [dict(shared) for _ in range(NCORES)]
    for pl in _POOLS:
        x, idx, W1, b1, W2, b2 = data[pl]
        xs, idxs = _prep_pool(x, idx, cfg[pl])
        W1 = np.asarray(W1, dtype=np.float32)
        W2 = np.concatenate([np.asarray(W2, dtype=np.float32).reshape(HALF, 1),
                             np.zeros((HALF, 1), np.float32)], axis=1)
        b1 = np.asarray(b1, dtype=np.float32).reshape(HALF, 1)
        b2 = np.broadcast_to(np.asarray(b2, dtype=np.float32).reshape(1, 1),
                             (P, 1)).copy()
        for c in range(NCORES):
            percore[c][f"x_{pl}"] = xs[c]
            percore[c][f"idx_{pl}"] = idxs[c]
            percore[c][f"w1_{pl}"] = W1
            percore[c][f"w2_{pl}"] = W2
            percore[c][f"b1_{pl}"] = b1
            percore[c][f"b2_{pl}"] = b2

    global _LAST
    _LAST = (nc, percore)
    if _PREP_ONLY:
        return None
    res = bass_utils.run_bass_kernel_spmd(nc, percore,
                                          core_ids=list(range(NCORES)))
    return np.asarray(res.results[0]["energy"], dtype=np.float32)


def prepare(inputs):
    """Build (or fetch cached) program + per-core input maps without running.
    For test harnesses that want to time execution separately."""
    global _PREP_ONLY
    _PREP_ONLY = True
    try:
        kernel(**inputs)
    finally:
        _PREP_ONLY = False
    return _LAST


_PREP_ONLY = False
_LAST = None

